# revision 30
# baseline (speedup 1.0000x reference)
"""Multi-head attention (B=4, S=2048, D=1024, H=16, dk=dv=64) on 8 Trainium2
NeuronCores.

Sharding: core c handles batch b = c//2 and head-group g = c%2 (8 of 16 heads).

Key structure (per core):
  - Scores via fp8e4m3 DoubleRow matmuls: qT is stored as (q8, dq8) residual
    pairs and kT duplicated (k8, k8), so one 0.5-cycle/row matmul computes
    (q8+dq8)@k8 -- near-bf16 accuracy on the Q side at 2x PE rate.
  - exp on the scalar engine only (the only engine with activation tables);
    everything else is kept off it.
  - PV in bf16 with a ones column appended to V so softmax row-sums fall out
    of the same matmuls.
  - All transposes (ctx^T -> natural, natural -> dv-major for o_proj) run on
    the DMA xbar (dma_start_transpose, bf16), not the PE.
  - Projections (K/Q/V) and o_proj are deferred work units popped inside the
    attention loops to fill PE slack under the scalar-engine exp stream.
  - Outputs are written bf16 (outT transposed, wts natural); the host sums
    the core pair for o_proj (row-parallel all-reduce) and upcasts.
"""
import sys

for _p in ("/opt/trn_rl_repo", "/root/.axon_site/_ro/trn_rl_repo"):
    if _p not in sys.path:
        sys.path.insert(0, _p)

import numpy as np
import ml_dtypes
import concourse.bass as bass
import concourse.bacc as bacc
import concourse.tile as tile
from concourse import mybir
from concourse.bass_utils import run_bass_kernel_spmd

F32 = mybir.dt.float32
BF16 = mybir.dt.bfloat16
FP8 = mybir.dt.float8e4
EXP = mybir.ActivationFunctionType.Exp
ADD = mybir.AluOpType.add
MULT = mybir.AluOpType.mult
DR = mybir.MatmulPerfMode.DoubleRow

NPBF16 = ml_dtypes.bfloat16

B, S, D = 4, 2048, 1024
H, DK, DV = 16, 64, 64
NCORES = 8
HC = H // 2          # heads per core
HDK = HC * DK        # 512 head dims per core
SQC = 512            # query-chunk width


def build_program(nc: bass.Bass, s=S, d=D, hc=HC):
    hdk = hc * DK
    ck_n = hdk // 128        # head pairs = 4
    dt_n = d // 128          # D contraction tiles = 8
    skt_n = s // 128         # key tiles = 16
    sq_n = s // SQC          # query chunks = 4
    zn = SQC // 128          # 128-row tiles per query chunk = 4

    xqt = nc.dram_tensor("xqt", [sq_n, 128, dt_n, SQC], BF16,
                         kind="ExternalInput")
    xkt = nc.dram_tensor("xkt", [sq_n, 128, dt_n, SQC], BF16,
                         kind="ExternalInput")
    xvt = nc.dram_tensor("xvt", [sq_n, 128, dt_n, SQC], BF16,
                         kind="ExternalInput")
    wq = nc.dram_tensor("wq", [128, dt_n, hdk], BF16, kind="ExternalInput")
    wk = nc.dram_tensor("wk", [128, dt_n, hdk], BF16, kind="ExternalInput")
    wv = nc.dram_tensor("wv", [128, dt_n, hdk], BF16, kind="ExternalInput")
    bqr = nc.dram_tensor("bqr", [1, hdk], BF16, kind="ExternalInput")
    bk = nc.dram_tensor("bk", [128, ck_n], F32, kind="ExternalInput")
    bv = nc.dram_tensor("bv", [1, hdk], BF16, kind="ExternalInput")
    wo = nc.dram_tensor("wo", [128, ck_n, d], BF16, kind="ExternalInput")
    bo = nc.dram_tensor("bo", [128, dt_n], F32, kind="ExternalInput")
    msk = nc.dram_tensor("msk", [128, skt_n], F32, kind="ExternalInput")

    outT_p = nc.dram_tensor("outT_p", [dt_n, sq_n, 128, SQC], BF16,
                            kind="ExternalOutput")
    wts_p = nc.dram_tensor("wts_p", [sq_n, ck_n, 128, zn, 128], BF16,
                           kind="ExternalOutput")
    wts_v = wts_p

    with tile.TileContext(nc) as tc, \
            tc.tile_pool(name="consts", bufs=1) as consts, \
            tc.tile_pool(name="persist", bufs=1) as persist:
        ident = consts.tile([128, 128], F32, name="ident")
        ident_bf = consts.tile([128, 128], BF16, name="ident_bf")
        ones1 = consts.tile([1, SQC], BF16, name="ones1")
        msk_sb = consts.tile([128, skt_n], F32, name="msk_sb")
        boT_sb = consts.tile([128, dt_n], F32, name="boT_sb")
        bqr_sb = consts.tile([1, hdk], BF16, name="bqr_sb")
        bv_sb = consts.tile([1, hdk], BF16, name="bv_sb")
        bk_t = consts.tile([128, ck_n], F32, name="bk_t")

        q8T = persist.tile([128, ck_n, 2, s], FP8, name="q8T")
        k8T = persist.tile([128, ck_n, 2, s], FP8, name="k8T")
        vtn = persist.tile([128, skt_n, hc, DV + 1], BF16, name="vtn")
        ones_th = consts.tile([128, skt_n * hc], BF16, name="ones_th")
        wo_sb = persist.tile([128, ck_n, d], BF16, name="wo_sb")
        bvb = persist.tile([128, hdk], BF16, name="bvb")

        from concourse.masks import make_identity
        make_identity(nc, ident)
        nc.vector.tensor_copy(ident_bf, ident)
        nc.gpsimd.memset(ones1, 1.0)
        nc.gpsimd.memset(ones_th, 1.0)
        nc.vector.tensor_copy(
            vtn[:, :, :, DV : DV + 1],
            ones_th.rearrange("p (t h one) -> p t h one", t=skt_n, one=1),
        )
        def load_consts():
            nc.sync.dma_start(out=bk_t, in_=bk[:])
            nc.sync.dma_start(out=bqr_sb, in_=bqr[:])
            nc.sync.dma_start(out=msk_sb, in_=msk[:])
            nc.sync.dma_start(out=boT_sb, in_=bo[:])
            nc.sync.dma_start(out=bv_sb, in_=bv[:])

        import contextlib
        with contextlib.ExitStack() as es:
            ec = es.enter_context
            xk_pool = ec(tc.tile_pool(name="xk", bufs=2))
            xq_pool = ec(tc.tile_pool(name="xq", bufs=2))
            xv_pool = ec(tc.tile_pool(name="xv", bufs=2))
            wz_pool = ec(tc.tile_pool(name="wz", bufs=3))
            ep_pool = ec(tc.tile_pool(name="ep", bufs=6))
            ctxu_pool = ec(tc.tile_pool(name="ctxu", bufs=3))
            nat_pool = ec(tc.tile_pool(name="nat", bufs=3))
            rs_pool = ec(tc.tile_pool(name="rs", bufs=3))
            rcp_pool = ec(tc.tile_pool(name="rcp", bufs=3))
            wnat_pool = ec(tc.tile_pool(name="wnat", bufs=3))
            wtsT_pool = ec(tc.tile_pool(name="wtsT", bufs=2))
            outsb_pool = ec(tc.tile_pool(name="outsb", bufs=6))
            part_pool = ec(tc.tile_pool(name="part", bufs=8))
            sc_ps = ec(tc.tile_pool(name="sc_ps", bufs=2, space="PSUM"))
            ctx_ps = ec(tc.tile_pool(name="ctx_ps", bufs=2, space="PSUM"))
            aux_ps = ec(tc.tile_pool(name="aux_ps", bufs=2, space="PSUM"))
            # begin body
            # ---------- helpers ----------
            def load_chunk(pool, xz, sb, eng=None):
                t_ = pool.tile([128, dt_n, SQC], BF16, name="xc_t")
                (eng or nc.gpsimd).dma_start(out=t_, in_=xz[sb])
                return t_

            def proj_k(xt_sb, ck, sb):
                """One [128, 512] chunk of K^T -> k8T (duplicated fp8)."""
                pp = aux_ps.tile([128, SQC], F32, name="aux")
                for dt_ in range(dt_n):
                    nc.tensor.matmul(
                        pp,
                        wk_sb[:, dt_, ck * 128 : (ck + 1) * 128],
                        xt_sb[:, dt_],
                        start=(dt_ == 0), stop=(dt_ == dt_n - 1),
                    )
                for slot in range(2):
                    nc.vector.tensor_scalar(
                        out=k8T[:, ck, slot, sb * SQC : (sb + 1) * SQC],
                        in0=pp, scalar1=bk_t[:, ck : ck + 1],
                        scalar2=None, op0=ADD,
                    )

            def proj_q(xt_sb, ck, sb):
                """One [128, 512] chunk of Q^T -> q8T (value+residual fp8).
                Bias is folded in via a ones-row matmul."""
                pp = aux_ps.tile([128, SQC], F32, name="aux")
                for dt_ in range(dt_n):
                    nc.tensor.matmul(
                        pp,
                        wq_sb[:, dt_, ck * 128 : (ck + 1) * 128],
                        xt_sb[:, dt_],
                        start=(dt_ == 0), stop=False,
                    )
                nc.tensor.matmul(
                    pp, bqr_sb[0:1, ck * 128 : (ck + 1) * 128], ones1,
                    start=False, stop=True,
                )
                dst0 = q8T[:, ck, 0, sb * SQC : (sb + 1) * SQC]
                nc.vector.tensor_copy(dst0, pp)
                nc.vector.scalar_tensor_tensor(
                    out=q8T[:, ck, 1, sb * SQC : (sb + 1) * SQC],
                    in0=dst0, scalar=-1.0, in1=pp,
                    op0=MULT, op1=ADD,
                )

            def proj_v(xt_sb, st):
                """One natural-layout [128 s, 512 e] V tile (s-tile st)."""
                vp = aux_ps.tile([128, hdk], F32, name="aux")
                stl = st % zn
                for dt_ in range(dt_n):
                    nc.tensor.matmul(
                        vp,
                        xt_sb[:, dt_, stl * 128 : (stl + 1) * 128],
                        wv_sb[:, dt_],
                        start=(dt_ == 0), stop=(dt_ == dt_n - 1),
                    )
                nc.vector.scalar_tensor_tensor(
                    out=vtn[:, st, :, 0:DV],
                    in0=vp.rearrange("p (h e) -> p h e", h=hc),
                    scalar=0.0,
                    in1=bvb.rearrange("p (h e) -> p h e", h=hc),
                    op0=mybir.AluOpType.bypass, op1=ADD,
                )

            # ---------- prologue ----------
            wk_sb = wz_pool.tile([128, dt_n, hdk], BF16, name="w_sb")
            nc.scalar.dma_start(out=wk_sb, in_=wk[:])
            xk_cur = load_chunk(xk_pool, xkt, 0)
            load_consts()
            xq_cur = load_chunk(xq_pool, xqt, 0)
            xv_cur = load_chunk(xv_pool, xvt, 0)
            wv_sb = wz_pool.tile([128, dt_n, hdk], BF16, name="w_sb")
            nc.gpsimd.dma_start(out=wv_sb, in_=wv[:])
            wq_sb = wz_pool.tile([128, dt_n, hdk], BF16, name="w_sb")
            nc.scalar.dma_start(out=wq_sb, in_=wq[:])
            nc.gpsimd.dma_start(out=wo_sb, in_=wo[:])

            for ck in range(ck_n):
                proj_k(xk_cur, ck, 0)
            proj_q(xq_cur, 0, 0)
            # V bias row broadcast: bvb[p, e] = bv[e]
            pbv = aux_ps.tile([128, hdk], F32, name="aux")
            nc.tensor.matmul(pbv, ones1[0:1, 0:128], bv_sb, start=True, stop=True)
            nc.vector.tensor_copy(bvb, pbv)
            proj_v(xv_cur, 0)

            # ---------- deferred work units ----------
            pending = []  # list of (deadline_in_global_tiles, emit_fn)
            holders = {"xk": {0: xk_cur}, "xq": {0: xq_cur}, "xv": {0: xv_cur}}

            def u_load_xk(sb):
                def emit():
                    holders["xk"][sb] = load_chunk(xk_pool, xkt, sb)
                return emit

            def u_proj_k(ck, sb):
                def emit():
                    proj_k(holders["xk"][sb], ck, sb)
                return emit

            def u_load_xv(sb):
                def emit():
                    holders["xv"][sb] = load_chunk(xv_pool, xvt, sb)
                return emit

            def u_proj_v(st):
                def emit():
                    proj_v(holders["xv"][st // zn], st)
                return emit

            def u_load_xq(qc):
                def emit():
                    holders["xq"][qc] = load_chunk(xq_pool, xqt, qc)
                return emit

            def u_proj_q(qc, ck):
                def emit():
                    proj_q(holders["xq"][qc], ck, qc)
                return emit

            # q0 prep: remaining K, V, Q0 with tile-index deadlines.
            # K is sb-major (compact): chunk sb's 4 units pop back-to-back so
            # the xk pool (bufs=2) never has 3 chunks in flight.
            for st in range(1, skt_n):
                sb = st // zn
                if st % zn == 0:
                    pending.append((4 * sb - 3.5, u_load_xv(sb)))
                pending.append((st - 1, u_proj_v(st)))
            for sb in range(1, sq_n):
                pending.append((5 * sb - 4.5, u_load_xk(sb)))
                for ck in range(ck_n):
                    pending.append((5 * sb - 4 + ck, u_proj_k(ck, sb)))
            for ck in range(1, ck_n):
                pending.append((16 * ck - 8, u_proj_q(0, ck)))

            def u_load_wo():
                def emit():
                    nc.gpsimd.dma_start(out=wo_sb, in_=wo[:])
                return emit
            pending.append((12, u_load_wo()))
            pending.sort(key=lambda x: x[0])

            def add_qproj(qc, base_dl):
                pending.append((base_dl, u_load_xq(qc)))
                for ck in range(ck_n):
                    pending.append((base_dl + 2 + 3 * ck, u_proj_q(qc, ck)))

            def oproj_chunk(q, dt_, wtsT_sb):
                def emit():
                    op = aux_ps.tile([128, SQC], F32, name="aux")
                    for et in range(ck_n):
                        nc.tensor.matmul(
                            op,
                            wo_sb[:, et, dt_ * 128 : (dt_ + 1) * 128],
                            wtsT_sb[:, et, :],
                            start=(et == 0), stop=(et == ck_n - 1),
                        )
                    out_sb = outsb_pool.tile([128, SQC], BF16, name="out_sb")
                    nc.vector.tensor_scalar(
                        out=out_sb, in0=op, scalar1=boT_sb[:, dt_ : dt_ + 1],
                        scalar2=None, op0=ADD,
                    )
                    nc.sync.dma_start(out=outT_p[dt_, q], in_=out_sb)
                return emit

            def oproj_part1(dt_, wtsT_sb, st):
                """First 3 of 4 contraction chunks; park the partial in SBUF."""
                def emit():
                    op = aux_ps.tile([128, SQC], F32, name="aux")
                    for et in range(ck_n - 1):
                        nc.tensor.matmul(
                            op,
                            wo_sb[:, et, dt_ * 128 : (dt_ + 1) * 128],
                            wtsT_sb[:, et, :],
                            start=(et == 0), stop=(et == ck_n - 2),
                        )
                    part_sb = part_pool.tile([128, SQC], F32, name="part_sb")
                    nc.vector.tensor_copy(part_sb, op)
                    st["part_sb"] = part_sb
                return emit

            def oproj_part2(q, dt_, wtsT_sb, st):
                def emit():
                    op = aux_ps.tile([128, SQC], F32, name="aux")
                    nc.tensor.matmul(
                        op,
                        wo_sb[:, ck_n - 1, dt_ * 128 : (dt_ + 1) * 128],
                        wtsT_sb[:, ck_n - 1, :],
                        start=True, stop=True,
                    )
                    out_sb = outsb_pool.tile([128, SQC], BF16, name="out_sb")
                    nc.vector.scalar_tensor_tensor(
                        out=out_sb, in0=op, scalar=boT_sb[:, dt_ : dt_ + 1],
                        in1=st["part_sb"], op0=ADD, op1=ADD,
                    )
                    eng = (nc.sync, nc.gpsimd, nc.scalar)[dt_ % 3]
                    eng.dma_start(out=outT_p[dt_, q], in_=out_sb)
                return emit

            # ---------- attention ----------
            for q in range(sq_n):
                q0 = q * SQC
                if q < sq_n - 1:
                    add_qproj(q + 1, 16 * ck_n * q + 8)
                    pending.sort(key=lambda x: x[0])
                wtsT_sb = wtsT_pool.tile([128, ck_n, SQC], BF16, name="wtsT_sb")
                for j in range(ck_n):
                    ctxA = ctx_ps.tile([DV + 1, SQC], F32, name="ctx_t")
                    ctxB = ctx_ps.tile([DV + 1, SQC], F32, name="ctx_t")
                    for t in range(skt_n):
                        gt = 64 * q + 16 * j + t
                        # scores(t) first; PV(t-1) emitted after, so its
                        # exp-wait hides under this tile's score matmuls.
                        sc = sc_ps.tile([128, 2 * SQC], F32, name="sc_t")
                        for m in range(2):
                            lo, hi = m * 64, (m + 1) * 64
                            nc.tensor.matmul(
                                sc[:, m * SQC : (m + 1) * SQC],
                                k8T[lo:hi, j, :, t * 128 : (t + 1) * 128],
                                q8T[lo:hi, j, :, q0 : q0 + SQC],
                                start=True, stop=True,
                                perf_mode=DR,
                                tile_position=(lo, 0),
                            )
                        ep = ep_pool.tile([128, 2 * SQC], BF16, name="ep_t")
                        nc.scalar.activation(
                            ep, sc, EXP, bias=msk_sb[:, t : t + 1], scale=0.125
                        )
                        # deferred work pops between scores(t) and PV(t-1)
                        pops = 0
                        while pending and (
                            pending[0][0] <= gt - 1
                            or (pops < 2 and pending[0][0] <= gt + 6)
                        ):
                            pending.pop(0)[1]()
                            pops += 1
                        if t > 0:
                            nc.tensor.matmul(
                                ctxA, vtn[:, t - 1, 2 * j], ep_pv[:, 0:SQC],
                                start=(t == 1), stop=False,
                            )
                            nc.tensor.matmul(
                                ctxB, vtn[:, t - 1, 2 * j + 1],
                                ep_pv[:, SQC : 2 * SQC],
                                start=(t == 1), stop=False,
                            )
                        ep_pv = ep
                    nc.tensor.matmul(
                        ctxA, vtn[:, skt_n - 1, 2 * j], ep_pv[:, 0:SQC],
                        start=False, stop=True,
                    )
                    nc.tensor.matmul(
                        ctxB, vtn[:, skt_n - 1, 2 * j + 1],
                        ep_pv[:, SQC : 2 * SQC],
                        start=False, stop=True,
                    )

                    # ---- post-block: normalize + transposes (PE, bf16) ----
                    ctxu = ctxu_pool.tile([DV + 1, 2 * SQC], BF16, name="ctxu_t")
                    nc.vector.tensor_copy(ctxu[:, 0:SQC], ctxA)
                    nc.vector.tensor_copy(ctxu[:, SQC : 2 * SQC], ctxB)
                    rc = rcp_pool.tile([128, 2, zn, 1], F32, name="rc_t")
                    wnat = wnat_pool.tile([128, zn, 128], BF16, name="wnat_t")
                    for m in range(2):
                        natp = aux_ps.tile([128, zn, DV + 1], F32, name="aux")
                        for zz in range(zn):
                            nc.tensor.matmul(
                                natp[:, zz],
                                ctxu[:, m * SQC + zz * 128 : m * SQC + (zz + 1) * 128],
                                ident_bf[0 : DV + 1, 0 : DV + 1],
                                start=True, stop=True,
                            )
                        nc.vector.reciprocal(rc[:, m], natp[:, :, DV : DV + 1])
                        for zz in range(zn):
                            nc.vector.tensor_scalar(
                                out=wnat[:, zz, m * DV : (m + 1) * DV],
                                in0=natp[:, zz, 0:DV],
                                scalar1=rc[:, m, zz],
                                scalar2=None,
                                op0=MULT,
                            )
                    nc.sync.dma_start(out=wts_v[q, j], in_=wnat)
                    # natural -> dv-major for o_proj: wnat^T via identity matmul
                    wtp = aux_ps.tile([128, zn, 128], F32, name="aux")
                    for m in range(2):
                        for zz in range(zn):
                            nc.tensor.matmul(
                                wtp[m * 64 : (m + 1) * 64, zz],
                                wnat[:, zz, m * 64 : (m + 1) * 64],
                                ident_bf,
                                start=True, stop=True,
                                tile_position=(0, m * 64),
                            )
                    nc.vector.tensor_copy(wtsT_sb[:, j], wtp)
                    if q == sq_n - 1 and j == ck_n - 2:
                        oproj_state = [dict() for _ in range(dt_n)]
                        for dt_ in range(dt_n):
                            pending.append((
                                64 * q + 16 * (j + 1) + 2 * dt_,
                                oproj_part1(dt_, wtsT_sb, oproj_state[dt_]),
                            ))
                        pending.sort(key=lambda x: x[0])
                if q == sq_n - 1:
                    for dt_ in range(dt_n):
                        oproj_part2(q, dt_, wtsT_sb, oproj_state[dt_])()
                else:
                    for dt_ in range(dt_n):
                        pending.append((
                            64 * q + 70 + 6 * dt_,
                            oproj_chunk(q, dt_, wtsT_sb),
                        ))
                    pending.sort(key=lambda x: x[0])
            while pending:
                pending.pop(0)[1]()
    return nc


_CACHE = {}


def _get_program():
    if "nc" not in _CACHE:
        nc = bacc.Bacc("TRN2")
        build_program(nc)
        nc.compile()
        _CACHE["nc"] = nc
    return _CACHE["nc"]


def kernel(query, key, value, mask, Wq, bq, Wk, bk, Wv, bv, Wo, bo, trace=False):
    f32 = lambda a: np.ascontiguousarray(a, dtype=np.float32)
    bf = lambda a: np.ascontiguousarray(np.asarray(a, dtype=np.float32), dtype=NPBF16)
    query, key, value, mask = map(np.asarray, (query, key, value, mask))
    Wq, bq, Wk, bk, Wv, bv, Wo, bo = map(f32, (Wq, bq, Wk, bk, Wv, bv, Wo, bo))
    zeros_bo = np.zeros_like(bo)

    def tile_x(x):
        # [S, D] -> x^T [D, S] -> [sb, 128p, 8t, 512s] chunk-contiguous
        xt = np.asarray(x, np.float32).T.reshape(8, 128, 4, 512)
        return bf(np.ascontiguousarray(xt.transpose(2, 1, 0, 3)))

    def tile_w(w):
        # [D, hdk] -> [128p, 8t, hdk]
        return bf(np.ascontiguousarray(
            np.asarray(w, np.float32).reshape(8, 128, HDK).transpose(1, 0, 2)))

    xT = {}
    for b in range(B):
        xT[b] = (tile_x(query[b]), tile_x(key[b]), tile_x(value[b]))

    in_maps = []
    for c in range(NCORES):
        b, g = c // 2, c % 2
        cols = slice(g * HDK, (g + 1) * HDK)
        xq_t, xk_t, xv_t = xT[b]
        in_maps.append({
            "xqt": xq_t, "xkt": xk_t, "xvt": xv_t,
            "wq": tile_w(Wq[:, cols]), "wk": tile_w(Wk[:, cols]),
            "wv": tile_w(Wv[:, cols]),
            "bqr": bf(bq[cols]).reshape(1, HDK),
            "bk": np.ascontiguousarray(
                bk[cols].reshape(HDK // 128, 128).T),
            "bv": bf(bv[cols]).reshape(1, HDK),
            "wo": bf(np.ascontiguousarray(
                Wo[cols, :].reshape(HDK // 128, 128, D).transpose(1, 0, 2))),
            "bo": np.ascontiguousarray(
                (bo if g == 0 else zeros_bo).reshape(D // 128, 128).T),
            "msk": np.ascontiguousarray(
                f32(mask[b, 0, 0]).reshape(S // 128, 128).T),
        })

    nc = _get_program()
    res = run_bass_kernel_spmd(
        nc, in_maps, core_ids=list(range(NCORES)), trace=trace
    )

    output = np.empty((B, S, D), np.float32)
    weights = np.empty((B, S, H * DV), np.float32)
    for b in range(B):
        # outT_p: [dt, q, p, c] -> out[s, d] = sum of core pair, transposed
        a0 = np.asarray(res.results[2 * b]["outT_p"], dtype=np.float32)
        a1 = np.asarray(res.results[2 * b + 1]["outT_p"], dtype=np.float32)
        a = a0 + a1  # [8, 4, 128, 512]
        output[b] = a.transpose(1, 3, 0, 2).reshape(S, D)
        # wts_p: [q, j, p, z, me] -> [s = q*512+z*128+p, j*128+me]
        for g, r in ((0, res.results[2 * b]), (1, res.results[2 * b + 1])):
            w = np.asarray(r["wts_p"], np.float32)  # [4, 4, 128, 4, 128]
            weights[b, :, g * HDK:(g + 1) * HDK] = (
                w.transpose(0, 3, 2, 1, 4).reshape(S, HDK))
    if trace:
        _CACHE["last_exec_time_ns"] = res.exec_time_ns
        _CACHE["last_res"] = res
    return output, weights


# revision 31
# speedup vs baseline: 1.0472x; 1.0472x over previous
"""Multi-head attention (B=4, S=2048, D=1024, H=16, dk=dv=64) on 8 Trainium2
NeuronCores.

Sharding: core c handles batch b = c//2 and head-group g = c%2 (8 of 16 heads).

Key structure (per core):
  - Scores via fp8e4m3 DoubleRow matmuls: qT is stored as (q8, dq8) residual
    pairs and kT duplicated (k8, k8), so one 0.5-cycle/row matmul computes
    (q8+dq8)@k8 -- near-bf16 accuracy on the Q side at 2x PE rate.
  - exp on the scalar engine only (the only engine with activation tables);
    everything else is kept off it.
  - PV in bf16 with a ones column appended to V so softmax row-sums fall out
    of the same matmuls.
  - All transposes (ctx^T -> natural, natural -> dv-major for o_proj) run on
    the DMA xbar (dma_start_transpose, bf16), not the PE.
  - Projections (K/Q/V) and o_proj are deferred work units popped inside the
    attention loops to fill PE slack under the scalar-engine exp stream.
  - Outputs are written bf16 (outT transposed, wts natural); the host sums
    the core pair for o_proj (row-parallel all-reduce) and upcasts.
"""
import sys

for _p in ("/opt/trn_rl_repo", "/root/.axon_site/_ro/trn_rl_repo"):
    if _p not in sys.path:
        sys.path.insert(0, _p)

import numpy as np
import ml_dtypes
import concourse.bass as bass
import concourse.bacc as bacc
import concourse.tile as tile
from concourse import mybir
from concourse.bass_utils import run_bass_kernel_spmd

F32 = mybir.dt.float32
BF16 = mybir.dt.bfloat16
FP8 = mybir.dt.float8e4
EXP = mybir.ActivationFunctionType.Exp
ADD = mybir.AluOpType.add
MULT = mybir.AluOpType.mult
DR = mybir.MatmulPerfMode.DoubleRow

NPBF16 = ml_dtypes.bfloat16

B, S, D = 4, 2048, 1024
H, DK, DV = 16, 64, 64
NCORES = 8
HC = H // 2          # heads per core
HDK = HC * DK        # 512 head dims per core
SQC = 512            # query-chunk width


def build_program(nc: bass.Bass, s=S, d=D, hc=HC):
    hdk = hc * DK
    ck_n = hdk // 128        # head pairs = 4
    dt_n = d // 128          # D contraction tiles = 8
    skt_n = s // 128         # key tiles = 16
    sq_n = s // SQC          # query chunks = 4
    zn = SQC // 128          # 128-row tiles per query chunk = 4

    xqt = nc.dram_tensor("xqt", [sq_n, 128, dt_n, SQC], BF16,
                         kind="ExternalInput")
    xkt = nc.dram_tensor("xkt", [sq_n, 128, dt_n, SQC], BF16,
                         kind="ExternalInput")
    xvt = nc.dram_tensor("xvt", [sq_n, 128, dt_n, SQC], BF16,
                         kind="ExternalInput")
    wq = nc.dram_tensor("wq", [128, dt_n, hdk], BF16, kind="ExternalInput")
    wk = nc.dram_tensor("wk", [128, dt_n, hdk], BF16, kind="ExternalInput")
    wv = nc.dram_tensor("wv", [128, dt_n, hdk], BF16, kind="ExternalInput")
    bqr = nc.dram_tensor("bqr", [1, hdk], BF16, kind="ExternalInput")
    bk = nc.dram_tensor("bk", [128, ck_n], F32, kind="ExternalInput")
    bv = nc.dram_tensor("bv", [1, hdk], BF16, kind="ExternalInput")
    wo = nc.dram_tensor("wo", [128, ck_n, d], BF16, kind="ExternalInput")
    bo = nc.dram_tensor("bo", [128, dt_n], F32, kind="ExternalInput")
    msk = nc.dram_tensor("msk", [128, skt_n], F32, kind="ExternalInput")

    outT_p = nc.dram_tensor("outT_p", [dt_n, sq_n, 128, SQC], BF16,
                            kind="ExternalOutput")
    wts_p = nc.dram_tensor("wts_p", [sq_n, ck_n, 128, zn, 128], BF16,
                           kind="ExternalOutput")
    wts_v = wts_p

    with tile.TileContext(nc) as tc, \
            tc.tile_pool(name="consts", bufs=1) as consts, \
            tc.tile_pool(name="persist", bufs=1) as persist:
        ident = consts.tile([128, 128], F32, name="ident")
        ident_bf = consts.tile([128, 128], BF16, name="ident_bf")
        ones1 = consts.tile([1, SQC], BF16, name="ones1")
        msk_sb = consts.tile([128, skt_n], F32, name="msk_sb")
        boT_sb = consts.tile([128, dt_n], F32, name="boT_sb")
        bqr_sb = consts.tile([1, hdk], BF16, name="bqr_sb")
        bv_sb = consts.tile([1, hdk], BF16, name="bv_sb")
        bk_t = consts.tile([128, ck_n], F32, name="bk_t")

        q8T = persist.tile([128, ck_n, 2, s], FP8, name="q8T")
        k8T = persist.tile([128, ck_n, 2, s], FP8, name="k8T")
        vtn = persist.tile([128, skt_n, hc, DV + 1], BF16, name="vtn")
        ones_th = consts.tile([128, skt_n * hc], BF16, name="ones_th")
        wo_sb = persist.tile([128, ck_n, d], BF16, name="wo_sb")
        bvb = persist.tile([128, hdk], BF16, name="bvb")

        from concourse.masks import make_identity
        make_identity(nc, ident)
        nc.vector.tensor_copy(ident_bf, ident)
        nc.gpsimd.memset(ones1, 1.0)
        nc.gpsimd.memset(ones_th, 1.0)
        nc.vector.tensor_copy(
            vtn[:, :, :, DV : DV + 1],
            ones_th.rearrange("p (t h one) -> p t h one", t=skt_n, one=1),
        )
        def load_consts():
            nc.sync.dma_start(out=bk_t, in_=bk[:])
            nc.sync.dma_start(out=bqr_sb, in_=bqr[:])
            nc.sync.dma_start(out=msk_sb, in_=msk[:])
            nc.sync.dma_start(out=boT_sb, in_=bo[:])
            nc.sync.dma_start(out=bv_sb, in_=bv[:])

        import contextlib
        with contextlib.ExitStack() as es:
            ec = es.enter_context
            xk_pool = ec(tc.tile_pool(name="xk", bufs=2))
            xq_pool = ec(tc.tile_pool(name="xq", bufs=2))
            xv_pool = ec(tc.tile_pool(name="xv", bufs=2))
            wz_pool = ec(tc.tile_pool(name="wz", bufs=3))
            ep_pool = ec(tc.tile_pool(name="ep", bufs=6))
            ctxu_pool = ec(tc.tile_pool(name="ctxu", bufs=3))
            nat_pool = ec(tc.tile_pool(name="nat", bufs=3))
            rs_pool = ec(tc.tile_pool(name="rs", bufs=3))
            rcp_pool = ec(tc.tile_pool(name="rcp", bufs=3))
            wnat_pool = ec(tc.tile_pool(name="wnat", bufs=3))
            wtsT_pool = ec(tc.tile_pool(name="wtsT", bufs=2))
            outsb_pool = ec(tc.tile_pool(name="outsb", bufs=6))
            part_pool = ec(tc.tile_pool(name="part", bufs=8))
            sc_ps = ec(tc.tile_pool(name="sc_ps", bufs=2, space="PSUM"))
            ctx_ps = ec(tc.tile_pool(name="ctx_ps", bufs=2, space="PSUM"))
            aux_ps = ec(tc.tile_pool(name="aux_ps", bufs=2, space="PSUM"))
            # begin body
            # ---------- helpers ----------
            def load_chunk(pool, xz, sb, eng=None):
                t_ = pool.tile([128, dt_n, SQC], BF16, name="xc_t")
                (eng or nc.gpsimd).dma_start(out=t_, in_=xz[sb])
                return t_

            def proj_k(xt_sb, ck, sb):
                """One [128, 512] chunk of K^T -> k8T (duplicated fp8)."""
                pp = aux_ps.tile([128, SQC], F32, name="aux")
                for dt_ in range(dt_n):
                    nc.tensor.matmul(
                        pp,
                        wk_sb[:, dt_, ck * 128 : (ck + 1) * 128],
                        xt_sb[:, dt_],
                        start=(dt_ == 0), stop=(dt_ == dt_n - 1),
                    )
                for slot in range(2):
                    nc.vector.tensor_scalar(
                        out=k8T[:, ck, slot, sb * SQC : (sb + 1) * SQC],
                        in0=pp, scalar1=bk_t[:, ck : ck + 1],
                        scalar2=None, op0=ADD,
                    )

            def proj_q(xt_sb, ck, sb):
                """One [128, 512] chunk of Q^T -> q8T (value+residual fp8).
                Bias is folded in via a ones-row matmul."""
                pp = aux_ps.tile([128, SQC], F32, name="aux")
                for dt_ in range(dt_n):
                    nc.tensor.matmul(
                        pp,
                        wq_sb[:, dt_, ck * 128 : (ck + 1) * 128],
                        xt_sb[:, dt_],
                        start=(dt_ == 0), stop=False,
                    )
                nc.tensor.matmul(
                    pp, bqr_sb[0:1, ck * 128 : (ck + 1) * 128], ones1,
                    start=False, stop=True,
                )
                dst0 = q8T[:, ck, 0, sb * SQC : (sb + 1) * SQC]
                nc.vector.tensor_copy(dst0, pp)
                nc.vector.scalar_tensor_tensor(
                    out=q8T[:, ck, 1, sb * SQC : (sb + 1) * SQC],
                    in0=dst0, scalar=-1.0, in1=pp,
                    op0=MULT, op1=ADD,
                )

            def proj_v(xt_sb, st):
                """One natural-layout [128 s, 512 e] V tile (s-tile st)."""
                vp = aux_ps.tile([128, hdk], F32, name="aux")
                stl = st % zn
                for dt_ in range(dt_n):
                    nc.tensor.matmul(
                        vp,
                        xt_sb[:, dt_, stl * 128 : (stl + 1) * 128],
                        wv_sb[:, dt_],
                        start=(dt_ == 0), stop=(dt_ == dt_n - 1),
                    )
                nc.vector.scalar_tensor_tensor(
                    out=vtn[:, st, :, 0:DV],
                    in0=vp.rearrange("p (h e) -> p h e", h=hc),
                    scalar=0.0,
                    in1=bvb.rearrange("p (h e) -> p h e", h=hc),
                    op0=mybir.AluOpType.bypass, op1=ADD,
                )

            # ---------- prologue ----------
            wk_sb = wz_pool.tile([128, dt_n, hdk], BF16, name="w_sb")
            nc.scalar.dma_start(out=wk_sb, in_=wk[:])
            xk_cur = load_chunk(xk_pool, xkt, 0)
            load_consts()
            xq_cur = load_chunk(xq_pool, xqt, 0)
            xv_cur = load_chunk(xv_pool, xvt, 0)
            wv_sb = wz_pool.tile([128, dt_n, hdk], BF16, name="w_sb")
            nc.gpsimd.dma_start(out=wv_sb, in_=wv[:])
            wq_sb = wz_pool.tile([128, dt_n, hdk], BF16, name="w_sb")
            nc.scalar.dma_start(out=wq_sb, in_=wq[:])
            nc.gpsimd.dma_start(out=wo_sb, in_=wo[:])

            for ck in range(ck_n):
                proj_k(xk_cur, ck, 0)
            proj_q(xq_cur, 0, 0)
            # V bias row broadcast: bvb[p, e] = bv[e]
            pbv = aux_ps.tile([128, hdk], F32, name="aux")
            nc.tensor.matmul(pbv, ones1[0:1, 0:128], bv_sb, start=True, stop=True)
            nc.vector.tensor_copy(bvb, pbv)
            proj_v(xv_cur, 0)

            # ---------- deferred work units ----------
            pending = []  # list of (deadline_in_global_tiles, emit_fn)
            holders = {"xk": {0: xk_cur}, "xq": {0: xq_cur}, "xv": {0: xv_cur}}

            def u_load_xk(sb):
                def emit():
                    holders["xk"][sb] = load_chunk(xk_pool, xkt, sb)
                return emit

            def u_proj_k(ck, sb):
                def emit():
                    proj_k(holders["xk"][sb], ck, sb)
                return emit

            def u_load_xv(sb):
                def emit():
                    holders["xv"][sb] = load_chunk(xv_pool, xvt, sb)
                return emit

            def u_proj_v(st):
                def emit():
                    proj_v(holders["xv"][st // zn], st)
                return emit

            def u_load_xq(qc):
                def emit():
                    holders["xq"][qc] = load_chunk(xq_pool, xqt, qc)
                return emit

            def u_proj_q(qc, ck):
                def emit():
                    proj_q(holders["xq"][qc], ck, qc)
                return emit

            # q0 prep: remaining K, V, Q0 with tile-index deadlines.
            # K is sb-major (compact): chunk sb's 4 units pop back-to-back so
            # the xk pool (bufs=2) never has 3 chunks in flight.
            for st in range(1, skt_n):
                sb = st // zn
                if st % zn == 0:
                    pending.append((4 * sb - 3.5, u_load_xv(sb)))
                pending.append((st - 1, u_proj_v(st)))
            for sb in range(1, sq_n):
                pending.append((5 * sb - 4.5, u_load_xk(sb)))
                for ck in range(ck_n):
                    pending.append((5 * sb - 4 + ck, u_proj_k(ck, sb)))
            for ck in range(1, ck_n):
                pending.append((16 * ck - 8, u_proj_q(0, ck)))

            def u_load_wo():
                def emit():
                    nc.gpsimd.dma_start(out=wo_sb, in_=wo[:])
                return emit
            pending.append((12, u_load_wo()))
            pending.sort(key=lambda x: x[0])

            def add_qproj(qc, base_dl):
                pending.append((base_dl, u_load_xq(qc)))
                for ck in range(ck_n):
                    pending.append((base_dl + 2 + 3 * ck, u_proj_q(qc, ck)))

            def oproj_chunk(q, dt_, wtsT_sb):
                def emit():
                    op = aux_ps.tile([128, SQC], F32, name="aux")
                    for et in range(ck_n):
                        nc.tensor.matmul(
                            op,
                            wo_sb[:, et, dt_ * 128 : (dt_ + 1) * 128],
                            wtsT_sb[:, et, :],
                            start=(et == 0), stop=(et == ck_n - 1),
                        )
                    out_sb = outsb_pool.tile([128, SQC], BF16, name="out_sb")
                    nc.vector.tensor_scalar(
                        out=out_sb, in0=op, scalar1=boT_sb[:, dt_ : dt_ + 1],
                        scalar2=None, op0=ADD,
                    )
                    nc.sync.dma_start(out=outT_p[dt_, q], in_=out_sb)
                return emit

            def oproj_part1(dt_, wtsT_sb, st):
                """First 3 of 4 contraction chunks; park the partial in SBUF."""
                def emit():
                    op = aux_ps.tile([128, SQC], F32, name="aux")
                    for et in range(ck_n - 1):
                        nc.tensor.matmul(
                            op,
                            wo_sb[:, et, dt_ * 128 : (dt_ + 1) * 128],
                            wtsT_sb[:, et, :],
                            start=(et == 0), stop=(et == ck_n - 2),
                        )
                    part_sb = part_pool.tile([128, SQC], F32, name="part_sb")
                    nc.vector.tensor_copy(part_sb, op)
                    st["part_sb"] = part_sb
                return emit

            def oproj_part2(q, dt_, wtsT_sb, st):
                def emit():
                    op = aux_ps.tile([128, SQC], F32, name="aux")
                    nc.tensor.matmul(
                        op,
                        wo_sb[:, ck_n - 1, dt_ * 128 : (dt_ + 1) * 128],
                        wtsT_sb[:, ck_n - 1, :],
                        start=True, stop=True,
                    )
                    out_sb = outsb_pool.tile([128, SQC], BF16, name="out_sb")
                    nc.vector.scalar_tensor_tensor(
                        out=out_sb, in0=op, scalar=boT_sb[:, dt_ : dt_ + 1],
                        in1=st["part_sb"], op0=ADD, op1=ADD,
                    )
                    eng = (nc.sync, nc.gpsimd, nc.scalar)[dt_ % 3]
                    eng.dma_start(out=outT_p[dt_, q], in_=out_sb)
                return emit

            # ---------- attention ----------
            for q in range(sq_n):
                q0 = q * SQC
                if q < sq_n - 1:
                    add_qproj(q + 1, 16 * ck_n * q + 8)
                    pending.sort(key=lambda x: x[0])
                wtsT_sb = wtsT_pool.tile([128, ck_n, SQC], BF16, name="wtsT_sb")
                for j in range(ck_n):
                    ctxA = ctx_ps.tile([DV + 1, SQC], F32, name="ctx_t")
                    ctxB = ctx_ps.tile([DV + 1, SQC], F32, name="ctx_t")
                    eps = {}
                    for t in range(skt_n):
                        gt = 64 * q + 16 * j + t
                        # scores(t) first; PV(t-1) emitted after, so its
                        # exp-wait hides under this tile's score matmuls.
                        sc = sc_ps.tile([128, 2 * SQC], F32, name="sc_t")
                        for m in range(2):
                            lo, hi = m * 64, (m + 1) * 64
                            nc.tensor.matmul(
                                sc[:, m * SQC : (m + 1) * SQC],
                                k8T[lo:hi, j, :, t * 128 : (t + 1) * 128],
                                q8T[lo:hi, j, :, q0 : q0 + SQC],
                                start=True, stop=True,
                                perf_mode=DR,
                                tile_position=(lo, 0),
                            )
                        ep = ep_pool.tile([128, 2 * SQC], BF16, name="ep_t")
                        nc.scalar.activation(
                            ep, sc, EXP, bias=msk_sb[:, t : t + 1], scale=0.125
                        )
                        # PV lagged 2 tiles: its exp input is long done, so
                        # the in-order PE stream never stalls on the ACT sem.
                        if t > 1:
                            nc.tensor.matmul(
                                ctxA, vtn[:, t - 2, 2 * j], eps[t - 2][:, 0:SQC],
                                start=(t == 2), stop=False,
                            )
                            nc.tensor.matmul(
                                ctxB, vtn[:, t - 2, 2 * j + 1],
                                eps[t - 2][:, SQC : 2 * SQC],
                                start=(t == 2), stop=False,
                            )
                        eps[t] = ep
                        # deferred work pops last (their own waits can't
                        # delay this tile's scores/PV)
                        pops = 0
                        while pending and (
                            pending[0][0] <= gt - 1
                            or (pops < 2 and pending[0][0] <= gt + 6)
                        ):
                            pending.pop(0)[1]()
                            pops += 1
                    for tt in (skt_n - 2, skt_n - 1):
                        nc.tensor.matmul(
                            ctxA, vtn[:, tt, 2 * j], eps[tt][:, 0:SQC],
                            start=False, stop=(tt == skt_n - 1),
                        )
                        nc.tensor.matmul(
                            ctxB, vtn[:, tt, 2 * j + 1],
                            eps[tt][:, SQC : 2 * SQC],
                            start=False, stop=(tt == skt_n - 1),
                        )

                    # ---- post-block: normalize + transposes (PE, bf16) ----
                    ctxu = ctxu_pool.tile([DV + 1, 2 * SQC], BF16, name="ctxu_t")
                    nc.vector.tensor_copy(ctxu[:, 0:SQC], ctxA)
                    nc.vector.tensor_copy(ctxu[:, SQC : 2 * SQC], ctxB)
                    rc = rcp_pool.tile([128, 2, zn, 1], F32, name="rc_t")
                    wnat = wnat_pool.tile([128, zn, 128], BF16, name="wnat_t")
                    for m in range(2):
                        natp = aux_ps.tile([128, zn, DV + 1], F32, name="aux")
                        for zz in range(zn):
                            nc.tensor.matmul(
                                natp[:, zz],
                                ctxu[:, m * SQC + zz * 128 : m * SQC + (zz + 1) * 128],
                                ident_bf[0 : DV + 1, 0 : DV + 1],
                                start=True, stop=True,
                            )
                        nc.vector.reciprocal(rc[:, m], natp[:, :, DV : DV + 1])
                        for zz in range(zn):
                            nc.vector.tensor_scalar(
                                out=wnat[:, zz, m * DV : (m + 1) * DV],
                                in0=natp[:, zz, 0:DV],
                                scalar1=rc[:, m, zz],
                                scalar2=None,
                                op0=MULT,
                            )
                    nc.sync.dma_start(out=wts_v[q, j], in_=wnat)
                    # natural -> dv-major for o_proj: wnat^T via identity matmul
                    wtp = aux_ps.tile([128, zn, 128], F32, name="aux")
                    for m in range(2):
                        for zz in range(zn):
                            nc.tensor.matmul(
                                wtp[m * 64 : (m + 1) * 64, zz],
                                wnat[:, zz, m * 64 : (m + 1) * 64],
                                ident_bf,
                                start=True, stop=True,
                                tile_position=(0, m * 64),
                            )
                    nc.vector.tensor_copy(wtsT_sb[:, j], wtp)
                    if q == sq_n - 1 and j == ck_n - 2:
                        oproj_state = [dict() for _ in range(dt_n)]
                        for dt_ in range(dt_n):
                            pending.append((
                                64 * q + 16 * (j + 1) + 2 * dt_,
                                oproj_part1(dt_, wtsT_sb, oproj_state[dt_]),
                            ))
                        pending.sort(key=lambda x: x[0])
                if q == sq_n - 1:
                    for dt_ in range(dt_n):
                        oproj_part2(q, dt_, wtsT_sb, oproj_state[dt_])()
                else:
                    for dt_ in range(dt_n):
                        pending.append((
                            64 * q + 70 + 6 * dt_,
                            oproj_chunk(q, dt_, wtsT_sb),
                        ))
                    pending.sort(key=lambda x: x[0])
            while pending:
                pending.pop(0)[1]()
    return nc


_CACHE = {}


def _get_program():
    if "nc" not in _CACHE:
        nc = bacc.Bacc("TRN2")
        build_program(nc)
        nc.compile()
        _CACHE["nc"] = nc
    return _CACHE["nc"]


def kernel(query, key, value, mask, Wq, bq, Wk, bk, Wv, bv, Wo, bo, trace=False):
    f32 = lambda a: np.ascontiguousarray(a, dtype=np.float32)
    bf = lambda a: np.ascontiguousarray(np.asarray(a, dtype=np.float32), dtype=NPBF16)
    query, key, value, mask = map(np.asarray, (query, key, value, mask))
    Wq, bq, Wk, bk, Wv, bv, Wo, bo = map(f32, (Wq, bq, Wk, bk, Wv, bv, Wo, bo))
    zeros_bo = np.zeros_like(bo)

    def tile_x(x):
        # [S, D] -> x^T [D, S] -> [sb, 128p, 8t, 512s] chunk-contiguous
        xt = np.asarray(x, np.float32).T.reshape(8, 128, 4, 512)
        return bf(np.ascontiguousarray(xt.transpose(2, 1, 0, 3)))

    def tile_w(w):
        # [D, hdk] -> [128p, 8t, hdk]
        return bf(np.ascontiguousarray(
            np.asarray(w, np.float32).reshape(8, 128, HDK).transpose(1, 0, 2)))

    xT = {}
    for b in range(B):
        xT[b] = (tile_x(query[b]), tile_x(key[b]), tile_x(value[b]))

    in_maps = []
    for c in range(NCORES):
        b, g = c // 2, c % 2
        cols = slice(g * HDK, (g + 1) * HDK)
        xq_t, xk_t, xv_t = xT[b]
        in_maps.append({
            "xqt": xq_t, "xkt": xk_t, "xvt": xv_t,
            "wq": tile_w(Wq[:, cols]), "wk": tile_w(Wk[:, cols]),
            "wv": tile_w(Wv[:, cols]),
            "bqr": bf(bq[cols]).reshape(1, HDK),
            "bk": np.ascontiguousarray(
                bk[cols].reshape(HDK // 128, 128).T),
            "bv": bf(bv[cols]).reshape(1, HDK),
            "wo": bf(np.ascontiguousarray(
                Wo[cols, :].reshape(HDK // 128, 128, D).transpose(1, 0, 2))),
            "bo": np.ascontiguousarray(
                (bo if g == 0 else zeros_bo).reshape(D // 128, 128).T),
            "msk": np.ascontiguousarray(
                f32(mask[b, 0, 0]).reshape(S // 128, 128).T),
        })

    nc = _get_program()
    res = run_bass_kernel_spmd(
        nc, in_maps, core_ids=list(range(NCORES)), trace=trace
    )

    output = np.empty((B, S, D), np.float32)
    weights = np.empty((B, S, H * DV), np.float32)
    for b in range(B):
        # outT_p: [dt, q, p, c] -> out[s, d] = sum of core pair, transposed
        a0 = np.asarray(res.results[2 * b]["outT_p"], dtype=np.float32)
        a1 = np.asarray(res.results[2 * b + 1]["outT_p"], dtype=np.float32)
        a = a0 + a1  # [8, 4, 128, 512]
        output[b] = a.transpose(1, 3, 0, 2).reshape(S, D)
        # wts_p: [q, j, p, z, me] -> [s = q*512+z*128+p, j*128+me]
        for g, r in ((0, res.results[2 * b]), (1, res.results[2 * b + 1])):
            w = np.asarray(r["wts_p"], np.float32)  # [4, 4, 128, 4, 128]
            weights[b, :, g * HDK:(g + 1) * HDK] = (
                w.transpose(0, 3, 2, 1, 4).reshape(S, HDK))
    if trace:
        _CACHE["last_exec_time_ns"] = res.exec_time_ns
        _CACHE["last_res"] = res
    return output, weights


# revision 32
# speedup vs baseline: 1.0498x; 1.0024x over previous
"""Multi-head attention (B=4, S=2048, D=1024, H=16, dk=dv=64) on 8 Trainium2
NeuronCores.

Sharding: core c handles batch b = c//2 and head-group g = c%2 (8 of 16 heads).

Key structure (per core):
  - Scores via fp8e4m3 DoubleRow matmuls: qT is stored as (q8, dq8) residual
    pairs and kT duplicated (k8, k8), so one 0.5-cycle/row matmul computes
    (q8+dq8)@k8 -- near-bf16 accuracy on the Q side at 2x PE rate.
  - exp on the scalar engine only (the only engine with activation tables);
    everything else is kept off it.
  - PV in bf16 with a ones column appended to V so softmax row-sums fall out
    of the same matmuls.
  - All transposes (ctx^T -> natural, natural -> dv-major for o_proj) run on
    the DMA xbar (dma_start_transpose, bf16), not the PE.
  - Projections (K/Q/V) and o_proj are deferred work units popped inside the
    attention loops to fill PE slack under the scalar-engine exp stream.
  - Outputs are written bf16 (outT transposed, wts natural); the host sums
    the core pair for o_proj (row-parallel all-reduce) and upcasts.
"""
import sys

for _p in ("/opt/trn_rl_repo", "/root/.axon_site/_ro/trn_rl_repo"):
    if _p not in sys.path:
        sys.path.insert(0, _p)

import numpy as np
import ml_dtypes
import concourse.bass as bass
import concourse.bacc as bacc
import concourse.tile as tile
from concourse import mybir
from concourse.bass_utils import run_bass_kernel_spmd

F32 = mybir.dt.float32
BF16 = mybir.dt.bfloat16
FP8 = mybir.dt.float8e4
EXP = mybir.ActivationFunctionType.Exp
ADD = mybir.AluOpType.add
MULT = mybir.AluOpType.mult
DR = mybir.MatmulPerfMode.DoubleRow

NPBF16 = ml_dtypes.bfloat16

B, S, D = 4, 2048, 1024
H, DK, DV = 16, 64, 64
NCORES = 8
HC = H // 2          # heads per core
HDK = HC * DK        # 512 head dims per core
SQC = 512            # query-chunk width


def build_program(nc: bass.Bass, s=S, d=D, hc=HC):
    hdk = hc * DK
    ck_n = hdk // 128        # head pairs = 4
    dt_n = d // 128          # D contraction tiles = 8
    skt_n = s // 128         # key tiles = 16
    sq_n = s // SQC          # query chunks = 4
    zn = SQC // 128          # 128-row tiles per query chunk = 4

    xqt = nc.dram_tensor("xqt", [sq_n, 128, dt_n, SQC], BF16,
                         kind="ExternalInput")
    xkt = nc.dram_tensor("xkt", [sq_n, 128, dt_n, SQC], BF16,
                         kind="ExternalInput")
    xvt = nc.dram_tensor("xvt", [sq_n, 128, dt_n, SQC], BF16,
                         kind="ExternalInput")
    wq = nc.dram_tensor("wq", [128, dt_n, hdk], BF16, kind="ExternalInput")
    wk = nc.dram_tensor("wk", [128, dt_n, hdk], BF16, kind="ExternalInput")
    wv = nc.dram_tensor("wv", [128, dt_n, hdk], BF16, kind="ExternalInput")
    bqr = nc.dram_tensor("bqr", [1, hdk], BF16, kind="ExternalInput")
    bk = nc.dram_tensor("bk", [128, ck_n], F32, kind="ExternalInput")
    bv = nc.dram_tensor("bv", [1, hdk], BF16, kind="ExternalInput")
    wo = nc.dram_tensor("wo", [128, ck_n, d], BF16, kind="ExternalInput")
    bo = nc.dram_tensor("bo", [128, dt_n], F32, kind="ExternalInput")
    msk = nc.dram_tensor("msk", [128, skt_n], F32, kind="ExternalInput")

    outT_p = nc.dram_tensor("outT_p", [dt_n, sq_n, 128, SQC], BF16,
                            kind="ExternalOutput")
    wts_p = nc.dram_tensor("wts_p", [sq_n, ck_n, 128, zn, 128], BF16,
                           kind="ExternalOutput")
    wts_v = wts_p

    with tile.TileContext(nc) as tc, \
            tc.tile_pool(name="consts", bufs=1) as consts, \
            tc.tile_pool(name="persist", bufs=1) as persist:
        ident = consts.tile([128, 128], F32, name="ident")
        ident_bf = consts.tile([128, 128], BF16, name="ident_bf")
        ones1 = consts.tile([1, SQC], BF16, name="ones1")
        msk_sb = consts.tile([128, skt_n], F32, name="msk_sb")
        boT_sb = consts.tile([128, dt_n], F32, name="boT_sb")
        bqr_sb = consts.tile([1, hdk], BF16, name="bqr_sb")
        bv_sb = consts.tile([1, hdk], BF16, name="bv_sb")
        bk_t = consts.tile([128, ck_n], F32, name="bk_t")

        q8T = persist.tile([128, ck_n, 2, s], FP8, name="q8T")
        k8T = persist.tile([128, ck_n, 2, s], FP8, name="k8T")
        vtn = persist.tile([128, skt_n, hc, DV + 1], BF16, name="vtn")
        ones_th = consts.tile([128, skt_n * hc], BF16, name="ones_th")
        wo_sb = persist.tile([128, ck_n, d], BF16, name="wo_sb")
        bvb = persist.tile([128, hdk], BF16, name="bvb")

        from concourse.masks import make_identity
        make_identity(nc, ident)
        nc.vector.tensor_copy(ident_bf, ident)
        nc.gpsimd.memset(ones1, 1.0)
        nc.gpsimd.memset(ones_th, 1.0)
        nc.vector.tensor_copy(
            vtn[:, :, :, DV : DV + 1],
            ones_th.rearrange("p (t h one) -> p t h one", t=skt_n, one=1),
        )
        def load_consts():
            nc.sync.dma_start(out=bk_t, in_=bk[:])
            nc.sync.dma_start(out=bqr_sb, in_=bqr[:])
            nc.sync.dma_start(out=msk_sb, in_=msk[:])
            nc.sync.dma_start(out=boT_sb, in_=bo[:])
            nc.sync.dma_start(out=bv_sb, in_=bv[:])

        import contextlib
        with contextlib.ExitStack() as es:
            ec = es.enter_context
            xk_pool = ec(tc.tile_pool(name="xk", bufs=2))
            xk0_pool = ec(tc.tile_pool(name="xk0", bufs=8))
            xq_pool = ec(tc.tile_pool(name="xq", bufs=2))
            xv_pool = ec(tc.tile_pool(name="xv", bufs=2))
            wz_pool = ec(tc.tile_pool(name="wz", bufs=3))
            ep_pool = ec(tc.tile_pool(name="ep", bufs=6))
            ctxu_pool = ec(tc.tile_pool(name="ctxu", bufs=3))
            nat_pool = ec(tc.tile_pool(name="nat", bufs=3))
            rs_pool = ec(tc.tile_pool(name="rs", bufs=3))
            rcp_pool = ec(tc.tile_pool(name="rcp", bufs=3))
            wnat_pool = ec(tc.tile_pool(name="wnat", bufs=3))
            wtsT_pool = ec(tc.tile_pool(name="wtsT", bufs=2))
            outsb_pool = ec(tc.tile_pool(name="outsb", bufs=6))
            part_pool = ec(tc.tile_pool(name="part", bufs=8))
            sc_ps = ec(tc.tile_pool(name="sc_ps", bufs=2, space="PSUM"))
            ctx_ps = ec(tc.tile_pool(name="ctx_ps", bufs=2, space="PSUM"))
            aux_ps = ec(tc.tile_pool(name="aux_ps", bufs=2, space="PSUM"))
            # begin body
            # ---------- helpers ----------
            def load_chunk(pool, xz, sb, eng=None):
                t_ = pool.tile([128, dt_n, SQC], BF16, name="xc_t")
                (eng or nc.gpsimd).dma_start(out=t_, in_=xz[sb])
                return t_

            def proj_k(xt_sb, ck, sb):
                """One [128, 512] chunk of K^T -> k8T (duplicated fp8)."""
                pp = aux_ps.tile([128, SQC], F32, name="aux")
                for dt_ in range(dt_n):
                    nc.tensor.matmul(
                        pp,
                        wk_sb[:, dt_, ck * 128 : (ck + 1) * 128],
                        xt_sb[:, dt_],
                        start=(dt_ == 0), stop=(dt_ == dt_n - 1),
                    )
                for slot in range(2):
                    nc.vector.tensor_scalar(
                        out=k8T[:, ck, slot, sb * SQC : (sb + 1) * SQC],
                        in0=pp, scalar1=bk_t[:, ck : ck + 1],
                        scalar2=None, op0=ADD,
                    )

            def proj_q(xt_sb, ck, sb):
                """One [128, 512] chunk of Q^T -> q8T (value+residual fp8).
                Bias is folded in via a ones-row matmul."""
                pp = aux_ps.tile([128, SQC], F32, name="aux")
                for dt_ in range(dt_n):
                    nc.tensor.matmul(
                        pp,
                        wq_sb[:, dt_, ck * 128 : (ck + 1) * 128],
                        xt_sb[:, dt_],
                        start=(dt_ == 0), stop=False,
                    )
                nc.tensor.matmul(
                    pp, bqr_sb[0:1, ck * 128 : (ck + 1) * 128], ones1,
                    start=False, stop=True,
                )
                dst0 = q8T[:, ck, 0, sb * SQC : (sb + 1) * SQC]
                nc.vector.tensor_copy(dst0, pp)
                nc.vector.scalar_tensor_tensor(
                    out=q8T[:, ck, 1, sb * SQC : (sb + 1) * SQC],
                    in0=dst0, scalar=-1.0, in1=pp,
                    op0=MULT, op1=ADD,
                )

            def proj_v(xt_sb, st):
                """One natural-layout [128 s, 512 e] V tile (s-tile st)."""
                vp = aux_ps.tile([128, hdk], F32, name="aux")
                stl = st % zn
                for dt_ in range(dt_n):
                    nc.tensor.matmul(
                        vp,
                        xt_sb[:, dt_, stl * 128 : (stl + 1) * 128],
                        wv_sb[:, dt_],
                        start=(dt_ == 0), stop=(dt_ == dt_n - 1),
                    )
                nc.vector.scalar_tensor_tensor(
                    out=vtn[:, st, :, 0:DV],
                    in0=vp.rearrange("p (h e) -> p h e", h=hc),
                    scalar=0.0,
                    in1=bvb.rearrange("p (h e) -> p h e", h=hc),
                    op0=mybir.AluOpType.bypass, op1=ADD,
                )

            # ---------- prologue ----------
            wk_sb = wz_pool.tile([128, dt_n, hdk], BF16, name="w_sb")
            nc.scalar.dma_start(out=wk_sb, in_=wk[:])
            xk_cur = load_chunk(xk_pool, xkt, 0)
            load_consts()
            xq_cur = load_chunk(xq_pool, xqt, 0)
            xv_cur = load_chunk(xv_pool, xvt, 0)
            wv_sb = wz_pool.tile([128, dt_n, hdk], BF16, name="w_sb")
            nc.gpsimd.dma_start(out=wv_sb, in_=wv[:])
            wq_sb = wz_pool.tile([128, dt_n, hdk], BF16, name="w_sb")
            nc.sync.dma_start(out=wq_sb, in_=wq[:])
            nc.gpsimd.dma_start(out=wo_sb, in_=wo[:])

            for ck in range(ck_n):
                proj_k(xk_cur, ck, 0)
            proj_q(xq_cur, 0, 0)
            # V bias row broadcast: bvb[p, e] = bv[e]
            pbv = aux_ps.tile([128, hdk], F32, name="aux")
            nc.tensor.matmul(pbv, ones1[0:1, 0:128], bv_sb, start=True, stop=True)
            nc.vector.tensor_copy(bvb, pbv)
            proj_v(xv_cur, 0)

            # ---------- deferred work units ----------
            pending = []  # list of (deadline_in_global_tiles, emit_fn)
            holders = {"xk": {0: xk_cur}, "xq": {0: xq_cur}, "xv": {0: xv_cur}}

            def u_load_xk(sb):
                def emit():
                    holders["xk"][sb] = load_chunk(xk_pool, xkt, sb)
                return emit

            def u_proj_k(ck, sb):
                def emit():
                    proj_k(holders["xk"][sb], ck, sb)
                return emit

            def u_load_xv(sb):
                def emit():
                    holders["xv"][sb] = load_chunk(xv_pool, xvt, sb)
                return emit

            def u_proj_v(st):
                def emit():
                    proj_v(holders["xv"][st // zn], st)
                return emit

            def u_load_xq(qc):
                def emit():
                    holders["xq"][qc] = load_chunk(xq_pool, xqt, qc)
                return emit

            def u_proj_q(qc, ck):
                def emit():
                    proj_q(holders["xq"][qc], ck, qc)
                return emit

            # q0 prep: remaining K, V, Q0 with tile-index deadlines.
            # K is sb-major (compact): chunk sb's 4 units pop back-to-back so
            # the xk pool (bufs=2) never has 3 chunks in flight.
            for st in range(1, skt_n):
                sb = st // zn
                if st % zn == 0:
                    pending.append((4 * sb - 3.5, u_load_xv(sb)))
                pending.append((st - 1, u_proj_v(st)))
            for sb in range(1, sq_n):
                pending.append((5 * sb - 4.5, u_load_xk(sb)))
                for ck in range(ck_n):
                    pending.append((5 * sb - 4 + ck, u_proj_k(ck, sb)))
            for ck in range(1, ck_n):
                pending.append((16 * ck - 8, u_proj_q(0, ck)))

            def u_load_wo():
                def emit():
                    nc.gpsimd.dma_start(out=wo_sb, in_=wo[:])
                return emit
            pending.append((12, u_load_wo()))
            pending.sort(key=lambda x: x[0])

            def add_qproj(qc, base_dl):
                pending.append((base_dl, u_load_xq(qc)))
                for ck in range(ck_n):
                    pending.append((base_dl + 2 + 3 * ck, u_proj_q(qc, ck)))

            def oproj_chunk(q, dt_, wtsT_sb):
                def emit():
                    op = aux_ps.tile([128, SQC], F32, name="aux")
                    for et in range(ck_n):
                        nc.tensor.matmul(
                            op,
                            wo_sb[:, et, dt_ * 128 : (dt_ + 1) * 128],
                            wtsT_sb[:, et, :],
                            start=(et == 0), stop=(et == ck_n - 1),
                        )
                    out_sb = outsb_pool.tile([128, SQC], BF16, name="out_sb")
                    nc.vector.tensor_scalar(
                        out=out_sb, in0=op, scalar1=boT_sb[:, dt_ : dt_ + 1],
                        scalar2=None, op0=ADD,
                    )
                    nc.sync.dma_start(out=outT_p[dt_, q], in_=out_sb)
                return emit

            def oproj_part1(dt_, wtsT_sb, st):
                """First 3 of 4 contraction chunks; park the partial in SBUF."""
                def emit():
                    op = aux_ps.tile([128, SQC], F32, name="aux")
                    for et in range(ck_n - 1):
                        nc.tensor.matmul(
                            op,
                            wo_sb[:, et, dt_ * 128 : (dt_ + 1) * 128],
                            wtsT_sb[:, et, :],
                            start=(et == 0), stop=(et == ck_n - 2),
                        )
                    part_sb = part_pool.tile([128, SQC], F32, name="part_sb")
                    nc.vector.tensor_copy(part_sb, op)
                    st["part_sb"] = part_sb
                return emit

            def oproj_part2(q, dt_, wtsT_sb, st):
                def emit():
                    op = aux_ps.tile([128, SQC], F32, name="aux")
                    nc.tensor.matmul(
                        op,
                        wo_sb[:, ck_n - 1, dt_ * 128 : (dt_ + 1) * 128],
                        wtsT_sb[:, ck_n - 1, :],
                        start=True, stop=True,
                    )
                    out_sb = outsb_pool.tile([128, SQC], BF16, name="out_sb")
                    nc.vector.scalar_tensor_tensor(
                        out=out_sb, in0=op, scalar=boT_sb[:, dt_ : dt_ + 1],
                        in1=st["part_sb"], op0=ADD, op1=ADD,
                    )
                    eng = (nc.sync, nc.gpsimd, nc.scalar)[dt_ % 3]
                    eng.dma_start(out=outT_p[dt_, q], in_=out_sb)
                return emit

            # ---------- attention ----------
            for q in range(sq_n):
                q0 = q * SQC
                if q < sq_n - 1:
                    add_qproj(q + 1, 16 * ck_n * q + 8)
                    pending.sort(key=lambda x: x[0])
                wtsT_sb = wtsT_pool.tile([128, ck_n, SQC], BF16, name="wtsT_sb")
                for j in range(ck_n):
                    ctxA = ctx_ps.tile([DV + 1, SQC], F32, name="ctx_t")
                    ctxB = ctx_ps.tile([DV + 1, SQC], F32, name="ctx_t")
                    eps = {}
                    for t in range(skt_n):
                        gt = 64 * q + 16 * j + t
                        # scores(t) first; PV(t-1) emitted after, so its
                        # exp-wait hides under this tile's score matmuls.
                        sc = sc_ps.tile([128, 2 * SQC], F32, name="sc_t")
                        for m in range(2):
                            lo, hi = m * 64, (m + 1) * 64
                            nc.tensor.matmul(
                                sc[:, m * SQC : (m + 1) * SQC],
                                k8T[lo:hi, j, :, t * 128 : (t + 1) * 128],
                                q8T[lo:hi, j, :, q0 : q0 + SQC],
                                start=True, stop=True,
                                perf_mode=DR,
                                tile_position=(lo, 0),
                            )
                        ep = ep_pool.tile([128, 2 * SQC], BF16, name="ep_t")
                        nc.scalar.activation(
                            ep, sc, EXP, bias=msk_sb[:, t : t + 1], scale=0.125
                        )
                        # PV lagged 2 tiles: its exp input is long done, so
                        # the in-order PE stream never stalls on the ACT sem.
                        if t > 1:
                            nc.tensor.matmul(
                                ctxA, vtn[:, t - 2, 2 * j], eps[t - 2][:, 0:SQC],
                                start=(t == 2), stop=False,
                            )
                            nc.tensor.matmul(
                                ctxB, vtn[:, t - 2, 2 * j + 1],
                                eps[t - 2][:, SQC : 2 * SQC],
                                start=(t == 2), stop=False,
                            )
                        eps[t] = ep
                        # deferred work pops last (their own waits can't
                        # delay this tile's scores/PV)
                        pops = 0
                        while pending and (
                            pending[0][0] <= gt - 1
                            or (pops < 2 and pending[0][0] <= gt + 6)
                        ):
                            pending.pop(0)[1]()
                            pops += 1
                    for tt in (skt_n - 2, skt_n - 1):
                        nc.tensor.matmul(
                            ctxA, vtn[:, tt, 2 * j], eps[tt][:, 0:SQC],
                            start=False, stop=(tt == skt_n - 1),
                        )
                        nc.tensor.matmul(
                            ctxB, vtn[:, tt, 2 * j + 1],
                            eps[tt][:, SQC : 2 * SQC],
                            start=False, stop=(tt == skt_n - 1),
                        )

                    # ---- post-block: normalize + transposes (PE, bf16) ----
                    ctxu = ctxu_pool.tile([DV + 1, 2 * SQC], BF16, name="ctxu_t")
                    nc.vector.tensor_copy(ctxu[:, 0:SQC], ctxA)
                    nc.vector.tensor_copy(ctxu[:, SQC : 2 * SQC], ctxB)
                    rc = rcp_pool.tile([128, 2, zn, 1], F32, name="rc_t")
                    wnat = wnat_pool.tile([128, zn, 128], BF16, name="wnat_t")
                    for m in range(2):
                        natp = aux_ps.tile([128, zn, DV + 1], F32, name="aux")
                        for zz in range(zn):
                            nc.tensor.matmul(
                                natp[:, zz],
                                ctxu[:, m * SQC + zz * 128 : m * SQC + (zz + 1) * 128],
                                ident_bf[0 : DV + 1, 0 : DV + 1],
                                start=True, stop=True,
                            )
                        nc.vector.reciprocal(rc[:, m], natp[:, :, DV : DV + 1])
                        for zz in range(zn):
                            nc.vector.tensor_scalar(
                                out=wnat[:, zz, m * DV : (m + 1) * DV],
                                in0=natp[:, zz, 0:DV],
                                scalar1=rc[:, m, zz],
                                scalar2=None,
                                op0=MULT,
                            )
                    nc.sync.dma_start(out=wts_v[q, j], in_=wnat)
                    # natural -> dv-major for o_proj: wnat^T via identity matmul
                    wtp = aux_ps.tile([128, zn, 128], F32, name="aux")
                    for m in range(2):
                        for zz in range(zn):
                            nc.tensor.matmul(
                                wtp[m * 64 : (m + 1) * 64, zz],
                                wnat[:, zz, m * 64 : (m + 1) * 64],
                                ident_bf,
                                start=True, stop=True,
                                tile_position=(0, m * 64),
                            )
                    nc.vector.tensor_copy(wtsT_sb[:, j], wtp)
                    if q == sq_n - 1 and j == ck_n - 2:
                        oproj_state = [dict() for _ in range(dt_n)]
                        for dt_ in range(dt_n):
                            pending.append((
                                64 * q + 16 * (j + 1) + 2 * dt_,
                                oproj_part1(dt_, wtsT_sb, oproj_state[dt_]),
                            ))
                        pending.sort(key=lambda x: x[0])
                if q == sq_n - 1:
                    for dt_ in range(dt_n):
                        oproj_part2(q, dt_, wtsT_sb, oproj_state[dt_])()
                else:
                    for dt_ in range(dt_n):
                        pending.append((
                            64 * q + 70 + 6 * dt_,
                            oproj_chunk(q, dt_, wtsT_sb),
                        ))
                    pending.sort(key=lambda x: x[0])
            while pending:
                pending.pop(0)[1]()
    return nc


_CACHE = {}


def _get_program():
    if "nc" not in _CACHE:
        nc = bacc.Bacc("TRN2")
        build_program(nc)
        nc.compile()
        _CACHE["nc"] = nc
    return _CACHE["nc"]


def kernel(query, key, value, mask, Wq, bq, Wk, bk, Wv, bv, Wo, bo, trace=False):
    f32 = lambda a: np.ascontiguousarray(a, dtype=np.float32)
    bf = lambda a: np.ascontiguousarray(np.asarray(a, dtype=np.float32), dtype=NPBF16)
    query, key, value, mask = map(np.asarray, (query, key, value, mask))
    Wq, bq, Wk, bk, Wv, bv, Wo, bo = map(f32, (Wq, bq, Wk, bk, Wv, bv, Wo, bo))
    zeros_bo = np.zeros_like(bo)

    def tile_x(x):
        # [S, D] -> x^T [D, S] -> [sb, 128p, 8t, 512s] chunk-contiguous
        xt = np.asarray(x, np.float32).T.reshape(8, 128, 4, 512)
        return bf(np.ascontiguousarray(xt.transpose(2, 1, 0, 3)))

    def tile_w(w):
        # [D, hdk] -> [128p, 8t, hdk]
        return bf(np.ascontiguousarray(
            np.asarray(w, np.float32).reshape(8, 128, HDK).transpose(1, 0, 2)))

    xT = {}
    for b in range(B):
        xT[b] = (tile_x(query[b]), tile_x(key[b]), tile_x(value[b]))

    in_maps = []
    for c in range(NCORES):
        b, g = c // 2, c % 2
        cols = slice(g * HDK, (g + 1) * HDK)
        xq_t, xk_t, xv_t = xT[b]
        in_maps.append({
            "xqt": xq_t, "xkt": xk_t, "xvt": xv_t,
            "wq": tile_w(Wq[:, cols]), "wk": tile_w(Wk[:, cols]),
            "wv": tile_w(Wv[:, cols]),
            "bqr": bf(bq[cols]).reshape(1, HDK),
            "bk": np.ascontiguousarray(
                bk[cols].reshape(HDK // 128, 128).T),
            "bv": bf(bv[cols]).reshape(1, HDK),
            "wo": bf(np.ascontiguousarray(
                Wo[cols, :].reshape(HDK // 128, 128, D).transpose(1, 0, 2))),
            "bo": np.ascontiguousarray(
                (bo if g == 0 else zeros_bo).reshape(D // 128, 128).T),
            "msk": np.ascontiguousarray(
                f32(mask[b, 0, 0]).reshape(S // 128, 128).T),
        })

    nc = _get_program()
    res = run_bass_kernel_spmd(
        nc, in_maps, core_ids=list(range(NCORES)), trace=trace
    )

    output = np.empty((B, S, D), np.float32)
    weights = np.empty((B, S, H * DV), np.float32)
    for b in range(B):
        # outT_p: [dt, q, p, c] -> out[s, d] = sum of core pair, transposed
        a0 = np.asarray(res.results[2 * b]["outT_p"], dtype=np.float32)
        a1 = np.asarray(res.results[2 * b + 1]["outT_p"], dtype=np.float32)
        a = a0 + a1  # [8, 4, 128, 512]
        output[b] = a.transpose(1, 3, 0, 2).reshape(S, D)
        # wts_p: [q, j, p, z, me] -> [s = q*512+z*128+p, j*128+me]
        for g, r in ((0, res.results[2 * b]), (1, res.results[2 * b + 1])):
            w = np.asarray(r["wts_p"], np.float32)  # [4, 4, 128, 4, 128]
            weights[b, :, g * HDK:(g + 1) * HDK] = (
                w.transpose(0, 3, 2, 1, 4).reshape(S, HDK))
    if trace:
        _CACHE["last_exec_time_ns"] = res.exec_time_ns
        _CACHE["last_res"] = res
    return output, weights
